# revision 23
# baseline (speedup 1.0000x reference)
"""Trainium2 Bass kernel for nn_Attention (dual-softmax linear attention), fp8 version.

Reference computation (per batch b):
  q  = x @ Wq                    [S, DM]   (DM = H*DH = 1024)
  kv = x @ Wkv                   [S, 2*DM] -> per head h: cols [h*128, h*128+64) = k_h,
                                              cols [h*128+64, (h+1)*128) = v_h
  q  = softmax(q over dh) * DH^-0.5
  k  = softmax(k over s)
  ctx_h   = k_h^T @ v_h          [DH, DH]
  out_h   = q_h @ ctx_h          [S, DH]
  y  = out @ Wlin + blin         [S, DM]

Sharding: data-parallel over batch B=8 -> one batch element per NeuronCore.

v3: all three big GEMMs (kv-proj, q-proj, final projection) run in fp8e4m3
with MatmulPerfMode.DoubleRow (2 k-planes per instruction, ~1.5x PE rate).
Numerics held together by three tricks (validated in numpy, rel err ~8e-4):
  1. Host-side bias correction: the dominant fp8 error is the common-mode
     shift of v's column means from quantizing Wv. y_corr = SCALE *
     ((xbar @ (Wv - Wv8)) @ Wlin) is computed on host in fp64 and folded
     into blin. (k/q softmax invariances kill the Wk/Wq quant errors.)
  2. Centered W2: the folded weight W2 = blockdiag(ctx_n)@Wlin*SCALE is
     nearly constant along each head's 64 contraction rows, so its fp8
     quantization error is rank-1 and large. The kernel computes per-head
     column means Kbar on device, subtracts them before quantizing
     (W2c = W2 - Kbar), routes sum_h Kbar through an exact fp32 bias path
     (valid because softmax rows sum to 1), and phase B contracts eq8@W2c.
     This also kills the eq8 quantization noise hitting the constant part.
  3. Scales: x*2^7, W*2^10 (products 2^17, descaled in the exp/copy
     activations), qhat*2^7 (blkones=2^-7), W2c*2^16 -> phase-B psum 2^23,
     output y*2^17 in fp16, descaled on host.

Layouts: x is transposed and quantized on HOST (xT [D, S] fp8), removing
all device-side transposes. DoubleRow operands are [128, 2, F] pair tiles.
"""

import math

import numpy as np

import concourse.bass as bass
import concourse.mybir as mybir
from concourse import bacc
from concourse.tile import TileContext

F32 = mybir.dt.float32
F16 = mybir.dt.float16
F8 = mybir.dt.float8e4
AF = mybir.ActivationFunctionType
DR = mybir.MatmulPerfMode.DoubleRow

S, D = 4096, 1024
H, DH = 16, 64
DM = H * DH  # 1024
B = 8
SCALE = DH ** (-0.5)

P = 128          # partitions
NB = 512         # moving free-dim tile
NP = D // (2 * P)  # 4 k-pair tiles
NJ = DM // P     # 8 dout-tiles (head pairs)
HH = H // 2      # heads per kv half-tile

SX = 2.0 ** 7    # x fp8 scale
SW = 2.0 ** 10   # weight fp8 scale
DESC = 2.0 ** -17  # product descale
SW2 = 2.0 ** 16  # centered-W2 fp8 scale
SY = 2.0 ** 17   # output scale (fp16 out, descaled on host)


def build_nc(s_len=S):
    sc = s_len // NB
    nc = bacc.Bacc(None, target_bir_lowering=False)

    xt_in = nc.declare_dram_parameter("xT", [D, s_len], F8, isOutput=False)
    wq_in = nc.declare_dram_parameter("Wq8", [D, DM], F8, isOutput=False)
    wkv_in = nc.declare_dram_parameter("Wkv8", [D, 2 * DM], F8, isOutput=False)
    wlin_in = nc.declare_dram_parameter("Wlin", [DM, DM], F16, isOutput=False)
    blin_in = nc.declare_dram_parameter("blin17", [1, DM], F32, isOutput=False)
    y_out = nc.declare_dram_parameter("y", [s_len, DM], F16, isOutput=True)

    with TileContext(nc) as tc:
        from contextlib import ExitStack

        with ExitStack() as stk:
            consts = stk.enter_context(tc.tile_pool(name="consts", bufs=1))
            wpool = stk.enter_context(tc.tile_pool(name="wpool", bufs=1))

            blkones = consts.tile([P, P], F16, tag="blkones")
            nc.vector.memset(blkones, 0.0)
            nc.vector.memset(blkones[0:64, 0:64], 2.0 ** -7)
            nc.vector.memset(blkones[64:128, 64:128], 2.0 ** -7)
            onescol = consts.tile([P, 1], F16, tag="onescol")
            nc.vector.memset(onescol, 1.0)
            # per-head-half mean-broadcast [P,P] blockdiag(1/64) and all-1/64
            blk64 = consts.tile([P, P], F16, tag="blk64")
            nc.vector.memset(blk64, 0.0)
            nc.vector.memset(blk64[0:64, 0:64], 1.0 / 64.0)
            nc.vector.memset(blk64[64:128, 64:128], 1.0 / 64.0)
            ones64 = consts.tile([P, P], F16, tag="ones64")
            nc.vector.memset(ones64, 1.0 / 64.0)

            # bias broadcast to all partitions via step-0 partition DMA
            bias_bc = consts.tile([P, DM], F32, tag="bias_bc")
            blin_row = blin_in[0, :]
            blin_bcast_ap = bass.AP(
                tensor=blin_row.tensor,
                offset=blin_row.offset,
                ap=[[0, P]] + list(blin_row.ap),
            )
            nc.gpsimd.dma_start(out=bias_bc, in_=blin_bcast_ap)

            # ctx / colsum accumulators (SBUF, fp32) + fp16 blockdiag staging
            bdt_tiles = []
            ctx_acc = []
            cs_acc = []
            for j in range(NJ):
                bdt = consts.tile([P, P], F16, tag=f"bdt{j}", name=f"bdt{j}")
                nc.vector.memset(bdt, 0.0)
                bdt_tiles.append(bdt)
                ca = consts.tile([P, P + 1], F32, tag=f"ctx_acc{j}")
                nc.vector.memset(ca, 0.0)
                ctx_acc.append(ca)
                cs_acc.append(ca[:, P:P + 1])

            # resident weights: fp8 pair layout [128, 2, cols]
            wkv_sb = [
                wpool.tile([P, 2, 2 * DM], F8, tag=f"wkv{p}", name=f"wkv{p}")
                for p in range(NP)
            ]
            wq_sb = [
                wpool.tile([P, 2, DM], F8, tag=f"wq{p}", name=f"wq{p}")
                for p in range(NP)
            ]
            wlin_sb = [
                wpool.tile([P, DM], F16, tag=f"wlin{j}", name=f"wlin{j}")
                for j in range(NJ)
            ]

            def load_weights():
                for p in range(NP):
                    for i in range(2):
                        r = (2 * p + i) * P
                        nc.sync.dma_start(
                            out=wq_sb[p][:, i, :], in_=wq_in[r:r + P, :]
                        )
                for h2 in range(2):
                    for p in range(NP):
                        for i in range(2):
                            r = (2 * p + i) * P
                            nc.sync.dma_start(
                                out=wkv_sb[p][:, i, h2 * DM:(h2 + 1) * DM],
                                in_=wkv_in[r:r + P, h2 * DM:(h2 + 1) * DM],
                            )
                for j in range(NJ):
                    nc.sync.dma_start(
                        out=wlin_sb[j], in_=wlin_in[j * P:(j + 1) * P, :]
                    )

            xt_pool = stk.enter_context(tc.tile_pool(name="xt", bufs=4))
            ek_pool = stk.enter_context(tc.tile_pool(name="ek", bufs=1))
            vt_pool = stk.enter_context(tc.tile_pool(name="vt", bufs=1))
            eq16_pool = stk.enter_context(tc.tile_pool(name="eq16", bufs=1))
            rr_pool = stk.enter_context(tc.tile_pool(name="rr", bufs=2))
            eqres_pool = stk.enter_context(tc.tile_pool(name="eqres", bufs=1))
            eq8_res = [[None] * NP for _ in range(sc)]

            # ---------------- phase A ----------------
            with (
                tc.tile_pool(name="kvp", bufs=2, space="PSUM") as kvp_pool,
                tc.tile_pool(name="ctxp", bufs=2, space="PSUM") as ctxp_pool,
                tc.tile_pool(name="qp", bufs=2, space="PSUM") as qp_pool,
            ):
                for c in range(sc):
                    xt = []
                    for p in range(NP):
                        t8 = xt_pool.tile([P, 2, NB], F8, tag=f"xt{p}")
                        for i in range(2):
                            r = (2 * p + i) * P
                            nc.sync.dma_start(
                                out=t8[:, i, :],
                                in_=xt_in[r:r + P, c * NB:(c + 1) * NB],
                            )
                        xt.append(t8)
                    if c == 0:
                        load_weights()

                    # q projection (DoubleRow fp8) -> eq8 = qhat * 2^7 (fp8)
                    # rowsum/rr/eq8 run one j behind the q-proj so the PE
                    # never waits on the eq16 ScalarE evac.
                    eq16_tiles = [None] * NJ

                    def flush_rowsum(j):
                        rsps = qp_pool.tile([P, NB], F32, tag="qp", name="rsps")
                        nc.tensor.matmul(rsps, blkones, eq16_tiles[j])
                        rr = rr_pool.tile([P, NB], F32, tag="rr", name="rr")
                        nc.vector.reciprocal_approx_fast(out=rr, in_=rsps)
                        jp, jo = j // 2, j % 2
                        if jo == 0:
                            eq8_res[c][jp] = eqres_pool.tile(
                                [P, 2, NB], F8, tag=f"eq{c}_{jp}",
                                name=f"eq{c}_{jp}",
                            )
                        nc.vector.tensor_mul(
                            eq8_res[c][jp][:, jo, :], eq16_tiles[j], rr
                        )

                    for j in range(NJ):
                        qps = qp_pool.tile([P, NB], F32, tag="qp", name="qps")
                        for p in range(NP):
                            nc.tensor.matmul(
                                qps,
                                wq_sb[p][:, :, j * P:(j + 1) * P],
                                xt[p],
                                start=(p == 0),
                                stop=(p == NP - 1),
                                perf_mode=DR,
                            )
                        eq16 = eq16_pool.tile(
                            [P, NB], F16, tag=f"eq16_{j % 3}", name="eq16"
                        )
                        nc.scalar.activation(eq16, qps, AF.Exp, scale=DESC)
                        eq16_tiles[j] = eq16
                        if j > 0:
                            flush_rowsum(j - 1)
                    flush_rowsum(NJ - 1)
                    # kv projection (DoubleRow fp8), two 1024-wide halves
                    ek_tiles = [[None, None] for _ in range(4)]
                    v_tiles = [[None, None] for _ in range(4)]
                    for t in range(4):
                        for h2 in range(2):
                            kvps = kvp_pool.tile([P, DM], F32, tag="kvp")
                            for p in range(NP):
                                for n in range(2):
                                    nc.tensor.matmul(
                                        kvps[:, n * NB:(n + 1) * NB],
                                        xt[p][:, :, t * P:(t + 1) * P],
                                        wkv_sb[p][
                                            :, :,
                                            h2 * DM + n * NB: h2 * DM + (n + 1) * NB,
                                        ],
                                        start=(p == 0),
                                        stop=(p == NP - 1),
                                        perf_mode=DR,
                                    )
                            kv3 = kvps.rearrange("p (h c) -> p h c", h=HH)
                            ek_t = ek_pool.tile([P, HH, DH], F16, tag=f"ek{t}_{h2}")
                            nc.scalar.activation(
                                ek_t, kv3[:, :, 0:DH], AF.Exp, scale=DESC
                            )
                            v_t = vt_pool.tile([P, HH, DH], F16, tag=f"v{t}_{h2}")
                            nc.vector.tensor_scalar(
                                out=v_t,
                                in0=kv3[:, :, DH:2 * DH],
                                scalar1=DESC,
                                scalar2=None,
                                op0=mybir.AluOpType.mult,
                            )
                            ek_tiles[t][h2] = ek_t.rearrange("p h c -> p (h c)")
                            v_tiles[t][h2] = v_t.rearrange("p h c -> p (h c)")

                    # ctx + colsum accumulation (per head-pair j), fp16
                    for j in range(NJ):
                        h2, jl = j // 4, j % 4
                        cps = ctxp_pool.tile([P, P + 4], F32, tag="ctxp")
                        for t in range(4):
                            nc.tensor.matmul(
                                cps[:, 0:P],
                                v_tiles[t][h2][:, jl * P:(jl + 1) * P],
                                ek_tiles[t][h2][:, jl * P:(jl + 1) * P],
                                start=(t == 0),
                                stop=False,
                            )
                            nc.tensor.matmul(
                                cps[:, P:P + 1],
                                ek_tiles[t][h2][:, jl * P:(jl + 1) * P],
                                onescol,
                                start=False,
                                stop=(t == 3),
                            )
                        nc.vector.tensor_add(
                            ctx_acc[j][:, 0:P + 1],
                            ctx_acc[j][:, 0:P + 1],
                            cps[:, 0:P + 1],
                        )


            # ---------------- fold: W2c (centered, fp8) + bias path ----------
            w2c8_sb = [None] * NP
            w2c8_pool = stk.enter_context(tc.tile_pool(name="w2c8", bufs=1))
            w2sb_pool = stk.enter_context(tc.tile_pool(name="w2sb", bufs=2))
            with (
                tc.tile_pool(name="w2p", bufs=2, space="PSUM") as w2p_pool,
                tc.tile_pool(name="kbbc", bufs=1, space="PSUM") as kbbc_pool,
                tc.tile_pool(name="ybp", bufs=1, space="PSUM") as ybp_pool,
            ):
                # sum_h Kbar, broadcast to all partitions (all-ones stationary)
                ybbc = ybp_pool.tile([P, DM], F32, tag="ybp")
                for j in range(NJ):
                    csr = consts.tile([P, 1], F32, tag=f"csr{j}")
                    nc.vector.tensor_scalar(
                        out=csr,
                        in0=cs_acc[j],
                        scalar1=1.0 / (SCALE * SW2),
                        scalar2=None,
                        op0=mybir.AluOpType.mult,
                    )
                    rcs = consts.tile([P, 1], F32, tag=f"rcs{j}")
                    nc.vector.reciprocal_approx_fast(out=rcs, in_=csr)
                    bdt = bdt_tiles[j]
                    nc.vector.tensor_copy(bdt[0:64, 0:64], ctx_acc[j][0:64, 0:64])
                    nc.vector.tensor_copy(
                        bdt[64:128, 64:128], ctx_acc[j][64:128, 64:128]
                    )
                    w2ps = w2p_pool.tile([P, DM], F32, tag="w2p")
                    for n in range(2):
                        nc.tensor.matmul(
                            w2ps[:, n * NB:(n + 1) * NB],
                            bdt,
                            wlin_sb[j][:, n * NB:(n + 1) * NB],
                        )
                    w2_16 = w2sb_pool.tile([P, DM], F16, tag="w2_16")
                    nc.scalar.activation(w2_16, w2ps, AF.Copy, scale=rcs)
                    for n in range(2):
                        nc.tensor.matmul(
                            ybbc[:, n * NB:(n + 1) * NB],
                            ones64,
                            w2_16[:, n * NB:(n + 1) * NB],
                            start=(j == 0),
                            stop=(j == NJ - 1),
                        )
                    kbbc = kbbc_pool.tile([P, DM], F32, tag="kbbc")
                    for n in range(2):
                        nc.tensor.matmul(
                            kbbc[:, n * NB:(n + 1) * NB],
                            blk64,
                            w2_16[:, n * NB:(n + 1) * NB],
                        )
                    jp, jo = j // 2, j % 2
                    if jo == 0:
                        w2c8_sb[jp] = w2c8_pool.tile(
                            [P, 2, DM], F8, tag=f"w2c{jp}", name=f"w2c{jp}"
                        )
                    nc.vector.scalar_tensor_tensor(
                        out=w2c8_sb[jp][:, jo, :],
                        in0=w2_16,
                        scalar=1.0,
                        in1=kbbc,
                        op0=mybir.AluOpType.mult,
                        op1=mybir.AluOpType.subtract,
                    )
                # bias = blin17_bc + 2 * ybbc  (2^17-scaled fp32)
                nc.vector.scalar_tensor_tensor(
                    out=bias_bc,
                    in0=ybbc,
                    scalar=2.0,
                    in1=bias_bc,
                    op0=mybir.AluOpType.mult,
                    op1=mybir.AluOpType.add,
                )

            y_pool = stk.enter_context(tc.tile_pool(name="ysb", bufs=3))

            # ---------------- phase B: final projection (DoubleRow fp8) ------
            with tc.tile_pool(name="yp", bufs=4, space="PSUM") as yp_pool:
                for c in range(sc):
                    for t in range(4):
                        yps = yp_pool.tile([P, DM], F32, tag="yp")
                        for jp in range(NP):
                            for n in range(2):
                                nc.tensor.matmul(
                                    yps[:, n * NB:(n + 1) * NB],
                                    eq8_res[c][jp][:, :, t * P:(t + 1) * P],
                                    w2c8_sb[jp][:, :, n * NB:(n + 1) * NB],
                                    start=(jp == 0),
                                    stop=(jp == NP - 1),
                                    perf_mode=DR,
                                )
                        ysb = y_pool.tile([P, DM], F16, tag="ysb")
                        nc.vector.scalar_tensor_tensor(
                            out=ysb,
                            in0=yps,
                            scalar=2.0 ** -6,
                            in1=bias_bc,
                            op0=mybir.AluOpType.mult,
                            op1=mybir.AluOpType.add,
                        )
                        nc.sync.dma_start(
                            out=y_out[c * NB + t * P: c * NB + (t + 1) * P, :],
                            in_=ysb,
                        )
    nc.compile()
    return nc


def _q8(a, scale):
    import ml_dtypes
    return np.clip(
        np.asarray(a, dtype=np.float32) * scale, -240.0, 240.0
    ).astype(ml_dtypes.float8_e4m3)


def prepare_inputs(x, Wq, Wkv, Wlin, blin):
    """Host-side quantization, transpose, and bias correction. Returns in_maps."""
    x = np.asarray(x, dtype=np.float32)
    b = x.shape[0]
    wq8 = _q8(Wq, SW)
    wkv8 = _q8(Wkv, SW)
    wlin16 = np.asarray(Wlin, dtype=np.float32).astype(np.float16)
    blin32 = np.asarray(blin, dtype=np.float64).reshape(DM)

    # host bias correction: dominant fp8 error is the common-mode shift of
    # v column means from quantizing Wv; exact to first order in fp64.
    vcols = np.concatenate(
        [np.arange(h * 2 * DH + DH, (h + 1) * 2 * DH) for h in range(H)]
    )
    Wkv64 = np.asarray(Wkv, dtype=np.float64)
    Wkv8_deq = wkv8.astype(np.float32).astype(np.float64) / SW
    dWv = Wkv64[:, vcols] - Wkv8_deq[:, vcols]          # [D, H*DH]
    xbar = x.astype(np.float64).mean(axis=1)            # [b, D]
    y_corr = SCALE * ((xbar @ dWv) @ np.asarray(Wlin, dtype=np.float64))

    in_maps = []
    for i in range(b):
        x8t = _q8(np.ascontiguousarray(x[i].T), SX)     # [D, S] fp8
        blin17 = ((blin32 + y_corr[i]) * SY).astype(np.float32).reshape(1, DM)
        in_maps.append(
            {
                "xT": x8t,
                "Wq8": wq8,
                "Wkv8": wkv8,
                "Wlin": wlin16,
                "blin17": blin17,
            }
        )
    return in_maps


def kernel(x, Wq, Wkv, Wlin, blin):
    from concourse.bass_utils import run_bass_kernel_spmd

    x = np.asarray(x, dtype=np.float32)
    b = x.shape[0]
    nc = build_nc(x.shape[1])
    in_maps = prepare_inputs(x, Wq, Wkv, Wlin, blin)
    res = run_bass_kernel_spmd(nc, in_maps, list(range(b)))
    return np.stack(
        [res.results[i]["y"].astype(np.float32) for i in range(b)]
    ) * np.float32(1.0 / SY)


if __name__ == "__main__":
    rng = np.random.default_rng(0)
    x = rng.random((B, S, D), dtype=np.float32)
    Wq = (rng.standard_normal((D, DM)) * 0.02).astype(np.float32)
    Wkv = (rng.standard_normal((D, 2 * DM)) * 0.02).astype(np.float32)
    Wlin = (rng.standard_normal((DM, DM)) * 0.02).astype(np.float32)
    blin = np.zeros((DM,), dtype=np.float32)
    y = kernel(x=x, Wq=Wq, Wkv=Wkv, Wlin=Wlin, blin=blin)
    print(y.shape, y.dtype)
